# revision 13
# baseline (speedup 1.0000x reference)
"""Trainium2 Bass kernel for nn_CausalityMapBlock.

Math: with p = 1.0 the [B,C,C,F*F] cross tensor collapses algebraically:
  sum_{i,j} (u_i v_j + e)^2 = S2u*S2v + 2e*S1u*S1v + e^2 F^2
  sum_{i,j} (u_i v_j + e)   = S1u*S1v + e F^2
so the whole block reduces to per-channel sums (S1, S2, S1a over F=49
spatial positions) followed by rank-1 outer products over the [C,C] grid.

With A1 = s*sum(x), A2 = s^2*sum(x^2), A1a = s*sum|x|, s = 1/(max+EPS):
  dd   = A1a + EPS*F            (lehmer denominator's denominator)
  nden = A2 + 2*EPS*A1a         (lehmer denominator's numerator)
  p    = nden + EPS*dd          (folds the +EPS after the ratio)
  out[m,n] = (A2[m]*A2[n]*dd[n] + 3*EPS*A1[m]*A1[n]*dd[n])
           / (A1[m]*A1[n]*p[n])
(constant terms ~1e-13 are >1000x below one fp32 ulp of the dominant
terms and are dropped). Numerator and denominator are rank-1 matmuls of
per-channel vectors; one reciprocal + one multiply finish the job.

All per-channel math runs in column layout [128,1] (full 128-lane DVE
parallelism); a single PE transpose moves the five final vectors to
[5,128] rows for the rank-1 matmuls.

Sharding: data-parallel over batch B=2; cores 0-3 compute batch 0,
cores 4-7 batch 1 (redundantly within a group; wall-clock identical).
"""

import sys

import numpy as np

for _p in ("/opt/trn_rl_repo",):
    if _p not in sys.path:
        sys.path.insert(0, _p)

EPS = 1e-8
B, C, H, W = 2, 128, 7, 7
F = H * W  # 49
N_CORES = 8

_CACHE = {}


def _build_nc():
    import concourse.bass as bass
    import concourse.bacc as bacc
    import concourse.mybir as mybir
    import concourse.tile as tile

    fp32 = mybir.dt.float32
    MUL = mybir.AluOpType.mult
    ADD = mybir.AluOpType.add
    # Bacc (not raw Bass): its compile() pass legalizes multi-wait
    # instructions, which this walrus build rejects at codegen otherwise.
    nc = bacc.Bacc("TRN2", target_bir_lowering=False, debug=False)
    xb = nc.dram_tensor("xb", [C, F], fp32, kind="ExternalInput")
    out = nc.dram_tensor("out", [C, C], fp32, kind="ExternalOutput")

    with tile.TileContext(nc) as tc:
        with (
            tc.tile_pool(name="sb", bufs=1) as sb,
            tc.tile_pool(name="ps", bufs=1, space=bass.MemorySpace.PSUM) as ps,
        ):
            ident = sb.tile([128, 128], fp32, tag="ident")
            nc.gpsimd.memset(ident[:], 0.0)
            nc.gpsimd.affine_select(
                out=ident[:], in_=ident[:],
                compare_op=mybir.AluOpType.not_equal,
                fill=1.0, base=0,
                pattern=[[-1, 128]], channel_multiplier=1,
            )
            ones_row = sb.tile([1, 128], fp32, tag="ones_row")
            nc.vector.memset(ones_row[:], 1.0)

            X = sb.tile([C, F], fp32, tag="X")
            nc.sync.dma_start(X[:], xb[:])

            # per-channel stats, column layout (128-lane parallel)
            mt = sb.tile([C, 1], fp32, tag="mt")
            s1c = sb.tile([C, 1], fp32, tag="s1c")
            s2c = sb.tile([C, 1], fp32, tag="s2c")
            s1ac = sb.tile([C, 1], fp32, tag="s1ac")
            X2 = sb.tile([C, F], fp32, tag="X2")
            nc.vector.reduce_max(mt[:], X[:], axis=mybir.AxisListType.X)
            nc.vector.reduce_sum(s1c[:], X[:], axis=mybir.AxisListType.X)
            nc.vector.scalar_tensor_tensor(
                X2[:], X[:], 1.0, X[:], op0=MUL, op1=MUL, accum_out=s2c[:],
            )
            nc.vector.reduce_sum(
                s1ac[:], X[:], axis=mybir.AxisListType.X,
                apply_absolute_value=True,
            )

            # global max on gpsimd (cross-partition all-reduce), then
            # s = 1/(max+EPS) computed lane-parallel in column layout
            from concourse import bass_isa

            g128 = sb.tile([C, 1], fp32, tag="g128")
            nc.gpsimd.partition_all_reduce(
                g128[:], mt[:], channels=C, reduce_op=bass_isa.ReduceOp.max,
            )
            sge = sb.tile([C, 1], fp32, tag="sge")
            sbc = sb.tile([C, 1], fp32, tag="sbc")
            nc.vector.tensor_scalar_add(sge[:], g128[:], float(EPS))
            nc.vector.reciprocal(sbc[:], sge[:])

            # scaled vectors + lehmer chain, all [128,1] columns.
            # V columns: 0=A1, 1=A2, 2=rhs1, 3=rhs0, 4=rhsD
            V = sb.tile([C, 8], fp32, tag="V")
            a1a = sb.tile([C, 1], fp32, tag="a1a")
            ddc = sb.tile([C, 1], fp32, tag="ddc")
            ndenc = sb.tile([C, 1], fp32, tag="ndenc")
            pc = sb.tile([C, 1], fp32, tag="pc")
            nc.vector.tensor_mul(V[:, 0:1], s1c[:], sbc[:])  # A1
            nc.vector.scalar_tensor_tensor(  # A2 = (S2r*s)*s
                V[:, 1:2], s2c[:], sbc[:], sbc[:], op0=MUL, op1=MUL,
            )
            nc.vector.tensor_mul(a1a[:], s1ac[:], sbc[:])  # A1a
            nc.vector.tensor_scalar_add(ddc[:], a1a[:], float(EPS * F))
            nc.vector.scalar_tensor_tensor(  # nden = A1a*2e + A2
                ndenc[:], a1a[:], float(2 * EPS), V[:, 1:2], op0=MUL, op1=ADD,
            )
            nc.vector.scalar_tensor_tensor(  # p = dd*e + nden
                pc[:], ddc[:], float(EPS), ndenc[:], op0=MUL, op1=ADD,
            )
            nc.vector.scalar_tensor_tensor(  # rhs1 = (A1*3e)*dd
                V[:, 2:3], V[:, 0:1], float(3 * EPS), ddc[:], op0=MUL, op1=MUL,
            )
            nc.vector.tensor_mul(V[:, 3:4], V[:, 1:2], ddc[:])  # rhs0
            nc.vector.tensor_mul(V[:, 4:5], V[:, 0:1], pc[:])  # rhsD

            # transpose to rows; matmul operands must sit at base partition
            # 0 of their own SBUF tiles (lhsT/rhs base must match & be 0)
            lt_ps = ps.tile([2, 128], fp32, tag="lt_ps")
            rt_ps = ps.tile([2, 128], fp32, tag="rt_ps")
            rd_ps = ps.tile([1, 128], fp32, tag="rd_ps")
            nc.tensor.transpose(lt_ps[:], V[:, 0:2], ident[:])  # [A1; A2]
            nc.tensor.transpose(rt_ps[:], V[:, 2:4], ident[:])  # [rhs1; rhs0]
            nc.tensor.transpose(rd_ps[:], V[:, 4:5], ident[:])  # [rhsD]
            LT = sb.tile([2, 128], fp32, tag="LT")
            RT = sb.tile([2, 256], fp32, tag="RT")
            nc.vector.memset(RT[:], 0.0)
            nc.vector.tensor_copy(LT[:], lt_ps[:])
            nc.scalar.copy(RT[:, 0:128], rt_ps[:])
            nc.vector.tensor_copy(RT[0:1, 128:256], rd_ps[:])

            # single K=2 N=256 matmul: cols 0-127 num = A1(x)rhs1 + A2(x)rhs0,
            # cols 128-255 den = A1(x)rhsD
            nd = ps.tile([128, 256], fp32, tag="nd")
            nc.tensor.matmul(nd[:], LT[:], RT[:], start=True, stop=True)

            # out = num * recip(den); inputs are benign positives so the
            # ~51-ulp fast reciprocal is far inside tolerance
            rden = sb.tile([128, 128], fp32, tag="rden")
            osb = sb.tile([128, 128], fp32, tag="osb")
            nc.vector.reciprocal_approx_fast(rden[:], nd[:, 128:256])
            nc.vector.tensor_mul(osb[:], nd[:, 0:128], rden[:])
            nc.sync.dma_start(out.ap(), osb[:])

    nc.compile()
    return nc


def _get_nc():
    if "nc" not in _CACHE:
        _CACHE["nc"] = _build_nc()
    return _CACHE["nc"]


def kernel(x) -> np.ndarray:
    from concourse.bass_utils import run_bass_kernel_spmd

    x = np.ascontiguousarray(np.asarray(x), dtype=np.float32)
    assert x.shape == (B, C, H, W)
    xf = x.reshape(B, C, F)

    nc = _get_nc()
    in_maps = [{"xb": np.ascontiguousarray(xf[i // 4])} for i in range(N_CORES)]
    res = run_bass_kernel_spmd(nc, in_maps, list(range(N_CORES))).results
    return np.stack([res[0]["out"], res[4]["out"]]).astype(np.float32)
